# revision 14
# baseline (speedup 1.0000x reference)
"""Trainium2 Bass kernel for nn_EngramConv: out = silu(dwconv(rmsnorm(x))) + x.

x [4, 4096, 2048] f32. Sharding: 8 cores, core i handles (batch i//2, half i%2)
= 2048 consecutive tokens (+ a 9-token halo supplying the causal-conv history;
host passes zeros at sequence start, so the kernel is branch-free SPMD).

v4 design, calibrated by on-device microbenchmarks (bench.py):
  - PE: ~30-40ns per 128x128 transpose (LDWEIGHTS overlaps back-to-back),
    ~220ns per 512-wide diag matmul when the stationary CHANGES (+125ns LDW
    penalty), ~95ns when it repeats -> conv processes tile PAIRS so each
    diag(w[c,k]) stationary is loaded once per two windows.
  - DVE: tensor_scalar bf16 4x, tensor_copy PSUM-bf16 2x, tensor_tensor 2x;
    scalar_tensor_tensor is ALWAYS 1x; f32 tensor_tensor 1x.
  - ACT ~584ns per [128,512] op; Pool bulk copies ~6x slower than DVE.
  - DMA: 16.8MB in + 16.8MB out = 87us on the SP queue alone, 78us when
    outputs go through Pool's SWDGE queue instead -> out-DMA on gpsimd.

Per-core pipeline over tile pairs (W = pair width = 768 or 512 tokens):
  DMA   x p-tile rows (layout 1, 8KB contiguous rows), SP queue
  ACT   Square+accum_out -> sum(x^2); scratch aimed at the xb arena, which
        the scaled cast overwrites right after
  DVE   Newton rsqrt -> rstd; scaled cast xb = x*rstd (f32->bf16, 2x_2p)
  PE    transpose-mode per (chunk, p-tile) -> PSUM bf16 (pairs of chunks)
  DVE   drain PSUM bf16 -> bufE[:, 2g:2g+2, 10+w*ts:...] (2x mode)
  Pool  9-token halo copy from the previous pair's bufE (one strided copy)
  PE    depthwise conv per (chunk, tap): stationary diag(w*norm) loaded
        once, then one accumulating matmul per window of the pair
  ACT   silu fused with the conv-PSUM drain -> bufE[:, c, w*ts:...] bf16
        (overwrites the spent conv input, disjoint from remaining reads)
  PE    transpose-mode back -> PSUM bf16
  DVE   residual add (+x f32) in place into x_t; out-DMA via gpsimd SWDGE

norm_weight is folded into the conv weights on the host (exact: depthwise
conv commutes with per-channel scaling).
"""

import numpy as np
import ml_dtypes

B, S, D = 4, 4096, 2048
KSZ, DIL = 4, 3
PAD = (KSZ - 1) * DIL  # 9
EPS = 1e-6
N_CORES = 8
TOKC = B * S // N_CORES  # 2048 tokens per core
P = 128
NCH = D // P              # 16 channel chunks

_cache = {}
ACT_NAME = "Silu"  # CoreSim has no Silu impl; HW does
TILE_SIZES = [384, 384, 384, 384, 256, 256]
CFG = {"t1_bufs": 2, "cv_bufs": 4, "t2_bufs": 2}


def _kernel_body(tc, out, x_main, x_halo, wdiag, ident, repeat=1):
    import concourse.bass as bass
    from concourse import mybir
    from contextlib import ExitStack, nullcontext

    nc = tc.nc
    f32 = mybir.dt.float32
    bf16 = mybir.dt.bfloat16
    AF = mybir.ActivationFunctionType
    AL = mybir.AluOpType

    with ExitStack() as ctx:
        consts = ctx.enter_context(tc.tile_pool(name="consts", bufs=1))
        xpool = ctx.enter_context(tc.tile_pool(name="xpool", bufs=3))
        xbpool = ctx.enter_context(tc.tile_pool(name="xbpool", bufs=2))
        xnt = ctx.enter_context(tc.tile_pool(name="xnt", bufs=2))
        small = ctx.enter_context(tc.tile_pool(name="small", bufs=8))
        ps_t1 = ctx.enter_context(
            tc.tile_pool(name="ps_t1", bufs=CFG["t1_bufs"], space="PSUM")
        )
        ps_cv = ctx.enter_context(
            tc.tile_pool(name="ps_cv", bufs=CFG["cv_bufs"], space="PSUM")
        )
        ps_t2 = ctx.enter_context(
            tc.tile_pool(name="ps_t2", bufs=CFG["t2_bufs"], space="PSUM")
        )

        # constants (outside the repeat loop)
        id_bf = consts.tile([P, P], bf16)
        nc.sync.dma_start(out=id_bf, in_=ident)
        w_sb = consts.tile([P, NCH, KSZ, P], bf16)
        nc.sync.dma_start(out=w_sb, in_=wdiag)
        eps_sb = consts.tile([P, 1], f32)
        nc.vector.memset(eps_sb, EPS)

        loop_cm = (
            tc.For_i(
                0, repeat, 1,
                hint_engines=(
                    mybir.EngineType.PE,
                    mybir.EngineType.Activation,
                    mybir.EngineType.DVE,
                    mybir.EngineType.Pool,
                    mybir.EngineType.SP,
                ),
            )
            if repeat > 1
            else nullcontext()
        )

        def make_rstd(ss_t, rstd_t):
            """rstd = 1/sqrt(m), m = ss/D + eps — DVE-only Newton iteration.

            m = mean(x^2) over D=2048 iid normal samples concentrates near 1,
            so a clamped linear seed + 3 Newton steps reaches fp32 accuracy
            for any plausible m; avoids ACT Sqrt (would force a LUT-set
            switch away from the silu table every tile).
            Zero rows (causal halo) give m=eps -> clamped seed; xn stays 0."""
            shp = [ss_t.shape[0], ss_t.shape[1]]
            m = small.tile(shp, f32, tag="nw_m", name="nw_m")
            nc.vector.tensor_scalar_mul(out=m, in0=ss_t, scalar1=1.0 / D)
            nc.vector.tensor_scalar_add(out=m, in0=m, scalar1=EPS)
            mc = small.tile(shp, f32, tag="nw_mc", name="nw_mc")
            nc.vector.tensor_scalar_max(out=mc, in0=m, scalar1=0.3)
            nc.vector.tensor_scalar_min(out=mc, in0=mc, scalar1=2.5)
            y = rstd_t
            nc.vector.tensor_scalar_mul(out=y, in0=mc, scalar1=-0.5)
            nc.vector.tensor_scalar_add(out=y, in0=y, scalar1=1.5)
            yy = small.tile(shp, f32, tag="nw_yy", name="nw_yy")
            t = small.tile(shp, f32, tag="nw_t", name="nw_t")
            for _ in range(3):
                nc.vector.tensor_mul(out=yy, in0=y, in1=y)
                nc.vector.scalar_tensor_tensor(
                    out=t, in0=yy, scalar=-0.5, in1=mc, op0=AL.mult, op1=AL.mult
                )
                nc.vector.tensor_scalar_add(out=t, in0=t, scalar1=1.5)
                nc.vector.tensor_mul(out=y, in0=t, in1=y)

        with loop_cm:
            tiles = TILE_SIZES
            assert sum(tiles) == TOKC
            assert len(tiles) % 2 == 0
            offs = [sum(tiles[:i]) for i in range(len(tiles))]
            npairs = len(tiles) // 2
            pre = {}
            pairbuf = {}

            def prelude_dma(it):
                ts = tiles[it]
                npt = ts // P
                t0 = offs[it]
                x_t = xpool.tile([P, npt, D], f32, tag="x", name=f"x{it}")
                for h in range(npt):
                    nc.sync.dma_start(
                        out=x_t[:, h:h + 1],
                        in_=x_main[t0 + h * P:t0 + (h + 1) * P, :].rearrange(
                            "(pt p) d -> p pt d", p=P
                        ),
                    )
                pre[("x", it)] = x_t

            def prelude(it):
                """stats (ACT Square scratched into the xb arena, DVE
                newton) and the scaled bf16 cast; allocates the pair buf
                when `it` is the first tile of its pair."""
                ts = tiles[it]
                npt = ts // P
                p = it // 2
                if it % 2 == 0:
                    W = tiles[2 * p] + tiles[2 * p + 1]
                    pairbuf[p] = xnt.tile(
                        [P, NCH, 10 + W], bf16, tag="bufE", name=f"bufE{p}"
                    )
                x_t = pre.pop(("x", it))
                xb = xbpool.tile([P, npt, D], bf16, tag="xb", name=f"xb{it}")
                ss_t = small.tile([P, npt], f32, tag="ss")
                for pt in range(npt):
                    # scratch target = this p-tile's xb slice, overwritten
                    # by the scaled cast right below
                    nc.scalar.activation(
                        out=xb[:, pt],
                        in_=x_t[:, pt],
                        func=AF.Square,
                        accum_out=ss_t[:, pt:pt + 1],
                    )
                rstd_t = small.tile([P, npt], f32, tag="rstd")
                make_rstd(ss_t, rstd_t)
                for pt in range(npt):
                    nc.vector.tensor_scalar_mul(
                        out=xb[:, pt], in0=x_t[:, pt],
                        scalar1=rstd_t[:, pt:pt + 1],
                    )
                pre[it] = (x_t, xb)

            def tin_drain(it):
                """transpose-mode to layout 2 (PSUM bf16, 2 chunks/tile)
                then the 2x DVE drain into the pair buffer."""
                ts = tiles[it]
                npt = ts // P
                p, w = it // 2, it % 2
                wbase = 10 + w * tiles[2 * p]
                bufE = pairbuf[p]
                x_t, xb = pre[it]
                for g in range(NCH // 2):
                    tpc = ps_t1.tile([P, 2, 512], bf16, tag="t1")
                    for ci in range(2):
                        c = 2 * g + ci
                        for pt in range(npt):
                            nc.tensor.transpose(
                                tpc[:, ci, pt * P:(pt + 1) * P],
                                xb[:, pt, c * P:(c + 1) * P],
                                id_bf,
                            )
                    nc.vector.tensor_copy(
                        out=bufE[:, 2 * g:2 * g + 2, wbase:wbase + ts],
                        in_=tpc[:, :, 0:ts],
                    )

            def pair_back(p):
                """conv (pair-amortized stationaries) + fused silu drain,
                then per-tile transpose-back, residual, out-DMA."""
                a, b = 2 * p, 2 * p + 1
                tsa, tsb = tiles[a], tiles[b]
                bufE = pairbuf.pop(p)
                # conv: stationary diag(w[c,k]) loaded once, two windows
                for c in range(NCH):
                    cvs = [
                        ps_cv.tile([P, 512], f32, tag="cv", name=f"cv{w}")
                        for w in range(2)
                    ]
                    for k in range(KSZ):
                        for w, ts in ((0, tsa), (1, tsb)):
                            nc.tensor.matmul(
                                cvs[w][:, 0:ts],
                                w_sb[:, c, k, :],
                                bufE[:, c,
                                     1 + 3 * k + w * tsa:
                                     1 + 3 * k + w * tsa + ts],
                                start=(k == 0),
                                stop=(k == KSZ - 1),
                            )
                    for w, ts in ((0, tsa), (1, tsb)):
                        nc.scalar.activation(
                            out=bufE[:, c, w * tsa:w * tsa + ts],
                            in_=cvs[w][:, 0:ts],
                            func=getattr(AF, ACT_NAME),
                        )
                # transpose back + residual + store, per tile of the pair
                HC = NCH // 2
                for w, it in ((0, a), (1, b)):
                    ts = tiles[it]
                    npt = ts // P
                    t0 = offs[it]
                    x_t, xb = pre.pop(it)
                    for pt in range(npt):
                        for hh in range(2):
                            op = ps_t2.tile([P, D // 2], bf16, tag="t2")
                            for ci in range(HC):
                                c = hh * HC + ci
                                nc.tensor.transpose(
                                    op[:, ci * P:(ci + 1) * P],
                                    bufE[:, c,
                                         w * tsa + pt * P:
                                         w * tsa + (pt + 1) * P],
                                    id_bf,
                                )
                            nc.vector.tensor_add(
                                out=x_t[:, pt, hh * (D // 2):(hh + 1) * (D // 2)],
                                in0=x_t[:, pt, hh * (D // 2):(hh + 1) * (D // 2)],
                                in1=op,
                            )
                        nc.gpsimd.dma_start(
                            out=out[t0 + pt * P:t0 + (pt + 1) * P, :].rearrange(
                                "(p one) d -> p one d", p=P
                            ),
                            in_=x_t[:, pt:pt + 1],
                        )
                return bufE

            prelude_dma(0)
            hx = small.tile([PAD, D], f32, tag="hx", name="hx", bufs=1)
            nc.sync.dma_start(out=hx, in_=x_halo)
            prelude(0)

            # ---- halo pre-tile: last PAD tokens feed tile 0's conv taps ----
            bufE0 = pairbuf[0]
            hss = small.tile([PAD, 1], f32, tag="hss", bufs=2)
            nc.scalar.activation(
                out=bufE0[0:PAD, 4:8, 10:10 + 512],
                in_=hx.rearrange("p (a b) -> p a b", a=4),
                func=AF.Square, accum_out=hss,
            )
            hrstd = small.tile([PAD, 1], f32, tag="hrstd", bufs=2)
            make_rstd(hss, hrstd)
            hxb = bufE0[0:PAD, 12:16, 10:10 + 512]
            nc.vector.tensor_scalar_mul(
                out=hxb, in0=hx.rearrange("p (a b) -> p a b", a=4),
                scalar1=hrstd,
            )
            ps_h = ps_t1.tile([P, NCH * 16], bf16, tag="t1")
            for c in range(NCH):
                nc.tensor.transpose(
                    ps_h[:, c * 16:c * 16 + PAD],
                    hxb[:, c // 4, (c % 4) * P:(c % 4 + 1) * P],
                    id_bf[0:PAD, 0:PAD],
                )
            nc.vector.tensor_copy(
                out=bufE0[:, :, 1:1 + PAD],
                in_=ps_h.rearrange("p (c h) -> p c h", c=NCH)[:, :, 0:PAD],
            )

            prelude_dma(1)
            prelude(1)

            tin_drain(0)
            tin_drain(1)
            for p in range(npairs):
                a, b = 2 * p, 2 * p + 1
                # next pair's DMA, stats, transposes and drains are emitted
                # BEFORE this pair's back phase so the list scheduler
                # overlaps them with the conv/silu/t-back tail
                if b + 1 < len(tiles):
                    prelude_dma(b + 1)
                    prelude(b + 1)
                if b + 2 < len(tiles):
                    prelude_dma(b + 2)
                    prelude(b + 2)
                if p + 1 < npairs:
                    W = tiles[a] + tiles[b]
                    # halo: last 9 tokens of this pair (Pool, tiny)
                    nc.gpsimd.tensor_copy(
                        out=pairbuf[p + 1][:, :, 1:1 + PAD],
                        in_=pairbuf[p][:, :, 1 + W:10 + W],
                    )
                    tin_drain(b + 1)
                    tin_drain(b + 2)
                pair_back(p)


def _build(repeat=1):
    if ("nc", repeat) in _cache:
        return _cache[("nc", repeat)]
    from concourse import bacc, mybir
    import concourse.tile as tile

    nc = bacc.Bacc(
        "TRN2",
        target_bir_lowering=False,
        debug=False,
        enable_asserts=False,
        num_devices=N_CORES,
    )
    f32 = mybir.dt.float32
    bf16 = mybir.dt.bfloat16
    x_main = nc.dram_tensor("x_main", [TOKC, D], f32, kind="ExternalInput").ap()
    x_halo = nc.dram_tensor("x_halo", [PAD, D], f32, kind="ExternalInput").ap()
    wdiag = nc.dram_tensor("wdiag", [P, NCH, KSZ, P], bf16, kind="ExternalInput").ap()
    ident = nc.dram_tensor("ident", [P, P], bf16, kind="ExternalInput").ap()
    out = nc.dram_tensor("out", [TOKC, D], f32, kind="ExternalOutput").ap()
    with tile.TileContext(nc) as tc:
        _kernel_body(tc, out, x_main, x_halo, wdiag, ident, repeat=repeat)
    nc.compile()
    _cache[("nc", repeat)] = nc
    return nc


def _make_in_maps(x, norm_weight, conv_weight):
    bf = ml_dtypes.bfloat16
    w = (conv_weight[:, 0, :] * norm_weight[:, None]).astype(np.float32)  # [D, K]
    wdiag = np.zeros((NCH, KSZ, P, P), np.float32)
    for c in range(NCH):
        for k in range(KSZ):
            np.fill_diagonal(wdiag[c, k], w[c * P:(c + 1) * P, k])
    wdiag = np.ascontiguousarray(wdiag.transpose(2, 0, 1, 3)).astype(bf)
    ident = np.eye(P, dtype=bf)
    zero_halo = np.zeros((PAD, D), np.float32)
    in_maps = []
    for core in range(N_CORES):
        b, h = core // 2, core % 2
        xm = np.ascontiguousarray(x[b, h * TOKC:(h + 1) * TOKC, :])
        xh = (
            np.ascontiguousarray(x[b, TOKC - PAD:TOKC, :]) if h == 1 else zero_halo
        )
        in_maps.append({"x_main": xm, "x_halo": xh, "wdiag": wdiag, "ident": ident})
    return in_maps


def _run(inputs, trace=False, repeat=1):
    from concourse import bass_utils

    nc = _build(repeat)
    in_maps = _make_in_maps(
        np.asarray(inputs["x"]),
        np.asarray(inputs["norm_weight"]),
        np.asarray(inputs["conv_weight"]),
    )
    kw = {}
    if trace:
        kw = dict(trace=True, trace_cores=list(range(N_CORES)))
    res = bass_utils.run_bass_kernel_spmd(
        nc, in_maps, core_ids=list(range(N_CORES)), **kw
    )
    outs = [res.results[i]["out"] for i in range(N_CORES)]
    full = np.stack(
        [np.concatenate([outs[2 * b], outs[2 * b + 1]], axis=0) for b in range(B)]
    )
    return full, res


def kernel(**inputs):
    full, _ = _run(inputs, trace=False)
    return full


# revision 15
# speedup vs baseline: 1.0968x; 1.0968x over previous
"""Trainium2 Bass kernel for nn_EngramConv: out = silu(dwconv(rmsnorm(x))) + x.

x [4, 4096, 2048] f32. Sharding: 8 cores, core i handles (batch i//2, half i%2)
= 2048 consecutive tokens (+ a 9-token halo supplying the causal-conv history;
host passes zeros at sequence start, so the kernel is branch-free SPMD).

v4 design, calibrated by on-device microbenchmarks (bench.py):
  - PE: ~30-40ns per 128x128 transpose (LDWEIGHTS overlaps back-to-back),
    ~220ns per 512-wide diag matmul when the stationary CHANGES (+125ns LDW
    penalty), ~95ns when it repeats -> conv processes tile PAIRS so each
    diag(w[c,k]) stationary is loaded once per two windows.
  - DVE: tensor_scalar bf16 4x, tensor_copy PSUM-bf16 2x, tensor_tensor 2x;
    scalar_tensor_tensor is ALWAYS 1x; f32 tensor_tensor 1x.
  - ACT ~584ns per [128,512] op; Pool bulk copies ~6x slower than DVE.
  - DMA: 16.8MB in + 16.8MB out = 87us on the SP queue alone, 78us when
    outputs go through Pool's SWDGE queue instead -> out-DMA on gpsimd.

Per-core pipeline over tile pairs (W = pair width = 768 or 512 tokens):
  DMA   x p-tile rows (layout 1, 8KB contiguous rows), SP queue
  ACT   Square+accum_out -> sum(x^2); scratch aimed at the xb arena, which
        the scaled cast overwrites right after
  DVE   Newton rsqrt -> rstd; scaled cast xb = x*rstd (f32->bf16, 2x_2p)
  PE    transpose-mode per (chunk, p-tile) -> PSUM bf16 (pairs of chunks)
  DVE   drain PSUM bf16 -> bufE[:, 2g:2g+2, 10+w*ts:...] (2x mode)
  Pool  9-token halo copy from the previous pair's bufE (one strided copy)
  PE    depthwise conv per (chunk, tap): stationary diag(w*norm) loaded
        once, then one accumulating matmul per window of the pair
  ACT   silu fused with the conv-PSUM drain -> bufE[:, c, w*ts:...] bf16
        (overwrites the spent conv input, disjoint from remaining reads)
  PE    transpose-mode back -> PSUM bf16
  DVE   residual add (+x f32) in place into x_t; out-DMA via gpsimd SWDGE

norm_weight is folded into the conv weights on the host (exact: depthwise
conv commutes with per-channel scaling).
"""

import numpy as np
import ml_dtypes

B, S, D = 4, 4096, 2048
KSZ, DIL = 4, 3
PAD = (KSZ - 1) * DIL  # 9
EPS = 1e-6
N_CORES = 8
TOKC = B * S // N_CORES  # 2048 tokens per core
P = 128
NCH = D // P              # 16 channel chunks

_cache = {}
ACT_NAME = "Silu"  # CoreSim has no Silu impl; HW does
TILE_SIZES = [384, 384, 384, 384, 256, 256]
CFG = {"t1_bufs": 2, "cv_bufs": 4, "t2_bufs": 2}


def _kernel_body(tc, out, x_main, x_halo, wdiag, ident, repeat=1):
    import concourse.bass as bass
    from concourse import mybir
    from contextlib import ExitStack, nullcontext

    nc = tc.nc
    f32 = mybir.dt.float32
    bf16 = mybir.dt.bfloat16
    AF = mybir.ActivationFunctionType
    AL = mybir.AluOpType

    with ExitStack() as ctx:
        consts = ctx.enter_context(tc.tile_pool(name="consts", bufs=1))
        xpool = ctx.enter_context(tc.tile_pool(name="xpool", bufs=3))
        xbpool = ctx.enter_context(tc.tile_pool(name="xbpool", bufs=2))
        xnt = ctx.enter_context(tc.tile_pool(name="xnt", bufs=2))
        small = ctx.enter_context(tc.tile_pool(name="small", bufs=8))
        ps_t1 = ctx.enter_context(
            tc.tile_pool(name="ps_t1", bufs=CFG["t1_bufs"], space="PSUM")
        )
        ps_cv = ctx.enter_context(
            tc.tile_pool(name="ps_cv", bufs=CFG["cv_bufs"], space="PSUM")
        )
        ps_t2 = ctx.enter_context(
            tc.tile_pool(name="ps_t2", bufs=CFG["t2_bufs"], space="PSUM")
        )

        # constants (outside the repeat loop)
        id_bf = consts.tile([P, P], bf16)
        nc.sync.dma_start(out=id_bf, in_=ident)
        w_sb = consts.tile([P, NCH, KSZ, P], bf16)
        nc.sync.dma_start(out=w_sb, in_=wdiag)
        eps_sb = consts.tile([P, 1], f32)
        nc.vector.memset(eps_sb, EPS)

        loop_cm = (
            tc.For_i(
                0, repeat, 1,
                hint_engines=(
                    mybir.EngineType.PE,
                    mybir.EngineType.Activation,
                    mybir.EngineType.DVE,
                    mybir.EngineType.Pool,
                    mybir.EngineType.SP,
                ),
            )
            if repeat > 1
            else nullcontext()
        )

        def make_rstd(ss_t, rstd_t):
            """rstd = 1/sqrt(m), m = ss/D + eps — DVE-only Newton iteration.

            m = mean(x^2) over D=2048 iid normal samples concentrates near 1,
            so a clamped linear seed + 3 Newton steps reaches fp32 accuracy
            for any plausible m; avoids ACT Sqrt (would force a LUT-set
            switch away from the silu table every tile).
            Zero rows (causal halo) give m=eps -> clamped seed; xn stays 0."""
            shp = [ss_t.shape[0], ss_t.shape[1]]
            m = small.tile(shp, f32, tag="nw_m", name="nw_m")
            nc.vector.tensor_scalar_mul(out=m, in0=ss_t, scalar1=1.0 / D)
            nc.vector.tensor_scalar_add(out=m, in0=m, scalar1=EPS)
            mc = small.tile(shp, f32, tag="nw_mc", name="nw_mc")
            nc.vector.tensor_scalar_max(out=mc, in0=m, scalar1=0.3)
            nc.vector.tensor_scalar_min(out=mc, in0=mc, scalar1=2.5)
            y = rstd_t
            nc.vector.tensor_scalar_mul(out=y, in0=mc, scalar1=-0.5)
            nc.vector.tensor_scalar_add(out=y, in0=y, scalar1=1.5)
            yy = small.tile(shp, f32, tag="nw_yy", name="nw_yy")
            t = small.tile(shp, f32, tag="nw_t", name="nw_t")
            for _ in range(3):
                nc.vector.tensor_mul(out=yy, in0=y, in1=y)
                nc.vector.scalar_tensor_tensor(
                    out=t, in0=yy, scalar=-0.5, in1=mc, op0=AL.mult, op1=AL.mult
                )
                nc.vector.tensor_scalar_add(out=t, in0=t, scalar1=1.5)
                nc.vector.tensor_mul(out=y, in0=t, in1=y)

        with loop_cm:
            tiles = TILE_SIZES
            assert sum(tiles) == TOKC
            assert len(tiles) % 2 == 0
            offs = [sum(tiles[:i]) for i in range(len(tiles))]
            npairs = len(tiles) // 2
            pre = {}
            pairbuf = {}

            def prelude_dma(it):
                ts = tiles[it]
                npt = ts // P
                t0 = offs[it]
                x_t = xpool.tile([P, npt, D], f32, tag="x", name=f"x{it}")
                for h in range(npt):
                    nc.sync.dma_start(
                        out=x_t[:, h:h + 1],
                        in_=x_main[t0 + h * P:t0 + (h + 1) * P, :].rearrange(
                            "(pt p) d -> p pt d", p=P
                        ),
                    )
                pre[("x", it)] = x_t

            def prelude(it):
                """stats (ACT Square scratched into the xb arena, DVE
                newton) and the scaled bf16 cast; allocates the pair buf
                when `it` is the first tile of its pair."""
                ts = tiles[it]
                npt = ts // P
                p = it // 2
                if it % 2 == 0:
                    W = tiles[2 * p] + tiles[2 * p + 1]
                    pairbuf[p] = xnt.tile(
                        [P, NCH, 10 + W], bf16, tag="bufE", name=f"bufE{p}"
                    )
                x_t = pre.pop(("x", it))
                xb = xbpool.tile([P, npt, D], bf16, tag="xb", name=f"xb{it}")
                ss_t = small.tile([P, npt], f32, tag="ss")
                for pt in range(npt):
                    # scratch target = this p-tile's xb slice, overwritten
                    # by the scaled cast right below
                    nc.scalar.activation(
                        out=xb[:, pt],
                        in_=x_t[:, pt],
                        func=AF.Square,
                        accum_out=ss_t[:, pt:pt + 1],
                    )
                rstd_t = small.tile([P, npt], f32, tag="rstd")
                make_rstd(ss_t, rstd_t)
                for pt in range(npt):
                    nc.vector.tensor_scalar_mul(
                        out=xb[:, pt], in0=x_t[:, pt],
                        scalar1=rstd_t[:, pt:pt + 1],
                    )
                pre[it] = (x_t, xb)

            def tin_drain(it):
                """transpose-mode to layout 2 (PSUM bf16, 2 chunks/tile)
                then the 2x DVE drain into the pair buffer."""
                ts = tiles[it]
                npt = ts // P
                p, w = it // 2, it % 2
                wbase = 10 + w * tiles[2 * p]
                bufE = pairbuf[p]
                x_t, xb = pre[it]
                for g in range(NCH // 2):
                    tpc = ps_t1.tile([P, 2, 512], bf16, tag="t1")
                    for ci in range(2):
                        c = 2 * g + ci
                        for pt in range(npt):
                            nc.tensor.transpose(
                                tpc[:, ci, pt * P:(pt + 1) * P],
                                xb[:, pt, c * P:(c + 1) * P],
                                id_bf,
                            )
                    nc.vector.tensor_copy(
                        out=bufE[:, 2 * g:2 * g + 2, wbase:wbase + ts],
                        in_=tpc[:, :, 0:ts],
                    )

            def pair_back(p):
                """conv (pair-amortized stationaries) + fused silu drain,
                then per-tile transpose-back, residual, out-DMA."""
                a, b = 2 * p, 2 * p + 1
                tsa, tsb = tiles[a], tiles[b]
                bufE = pairbuf.pop(p)
                # conv: stationary diag(w[c,k]) loaded once, two windows
                for c in range(NCH):
                    cvs = [
                        ps_cv.tile([P, 512], f32, tag="cv", name=f"cv{w}")
                        for w in range(2)
                    ]
                    for k in range(KSZ):
                        for w, ts in ((0, tsa), (1, tsb)):
                            nc.tensor.matmul(
                                cvs[w][:, 0:ts],
                                w_sb[:, c, k, :],
                                bufE[:, c,
                                     1 + 3 * k + w * tsa:
                                     1 + 3 * k + w * tsa + ts],
                                start=(k == 0),
                                stop=(k == KSZ - 1),
                            )
                    for w, ts in ((0, tsa), (1, tsb)):
                        nc.scalar.activation(
                            out=bufE[:, c, w * tsa:w * tsa + ts],
                            in_=cvs[w][:, 0:ts],
                            func=getattr(AF, ACT_NAME),
                        )
                # transpose back + residual + store, per tile of the pair
                HC = NCH // 2
                for w, it in ((0, a), (1, b)):
                    ts = tiles[it]
                    npt = ts // P
                    t0 = offs[it]
                    x_t, xb = pre.pop(it)
                    for pt in range(npt):
                        for hh in range(2):
                            op = ps_t2.tile([P, D // 2], bf16, tag="t2")
                            for ci in range(HC):
                                c = hh * HC + ci
                                nc.tensor.transpose(
                                    op[:, ci * P:(ci + 1) * P],
                                    bufE[:, c,
                                         w * tsa + pt * P:
                                         w * tsa + (pt + 1) * P],
                                    id_bf,
                                )
                            nc.vector.tensor_add(
                                out=x_t[:, pt, hh * (D // 2):(hh + 1) * (D // 2)],
                                in0=x_t[:, pt, hh * (D // 2):(hh + 1) * (D // 2)],
                                in1=op,
                            )
                        nc.sync.dma_start(
                            out=out[t0 + pt * P:t0 + (pt + 1) * P, :].rearrange(
                                "(p one) d -> p one d", p=P
                            ),
                            in_=x_t[:, pt:pt + 1],
                        )
                return bufE

            prelude_dma(0)
            hx = small.tile([PAD, D], f32, tag="hx", name="hx", bufs=1)
            nc.sync.dma_start(out=hx, in_=x_halo)
            prelude(0)

            # ---- halo pre-tile: last PAD tokens feed tile 0's conv taps ----
            bufE0 = pairbuf[0]
            hss = small.tile([PAD, 1], f32, tag="hss", bufs=2)
            nc.scalar.activation(
                out=bufE0[0:PAD, 4:8, 10:10 + 512],
                in_=hx.rearrange("p (a b) -> p a b", a=4),
                func=AF.Square, accum_out=hss,
            )
            hrstd = small.tile([PAD, 1], f32, tag="hrstd", bufs=2)
            make_rstd(hss, hrstd)
            hxb = bufE0[0:PAD, 12:16, 10:10 + 512]
            nc.vector.tensor_scalar_mul(
                out=hxb, in0=hx.rearrange("p (a b) -> p a b", a=4),
                scalar1=hrstd,
            )
            ps_h = ps_t1.tile([P, NCH * 16], bf16, tag="t1")
            for c in range(NCH):
                nc.tensor.transpose(
                    ps_h[:, c * 16:c * 16 + PAD],
                    hxb[:, c // 4, (c % 4) * P:(c % 4 + 1) * P],
                    id_bf[0:PAD, 0:PAD],
                )
            nc.vector.tensor_copy(
                out=bufE0[:, :, 1:1 + PAD],
                in_=ps_h.rearrange("p (c h) -> p c h", c=NCH)[:, :, 0:PAD],
            )

            prelude_dma(1)
            prelude(1)

            tin_drain(0)
            tin_drain(1)
            for p in range(npairs):
                a, b = 2 * p, 2 * p + 1
                # next pair's DMA, stats, transposes and drains are emitted
                # BEFORE this pair's back phase so the list scheduler
                # overlaps them with the conv/silu/t-back tail
                if b + 1 < len(tiles):
                    prelude_dma(b + 1)
                    prelude(b + 1)
                if b + 2 < len(tiles):
                    prelude_dma(b + 2)
                    prelude(b + 2)
                if p + 1 < npairs:
                    W = tiles[a] + tiles[b]
                    # halo: last 9 tokens of this pair (Pool, tiny)
                    nc.gpsimd.tensor_copy(
                        out=pairbuf[p + 1][:, :, 1:1 + PAD],
                        in_=pairbuf[p][:, :, 1 + W:10 + W],
                    )
                    tin_drain(b + 1)
                    tin_drain(b + 2)
                pair_back(p)


def _build(repeat=1):
    if ("nc", repeat) in _cache:
        return _cache[("nc", repeat)]
    from concourse import bacc, mybir
    import concourse.tile as tile

    nc = bacc.Bacc(
        "TRN2",
        target_bir_lowering=False,
        debug=False,
        enable_asserts=False,
        num_devices=N_CORES,
    )
    f32 = mybir.dt.float32
    bf16 = mybir.dt.bfloat16
    x_main = nc.dram_tensor("x_main", [TOKC, D], f32, kind="ExternalInput").ap()
    x_halo = nc.dram_tensor("x_halo", [PAD, D], f32, kind="ExternalInput").ap()
    wdiag = nc.dram_tensor("wdiag", [P, NCH, KSZ, P], bf16, kind="ExternalInput").ap()
    ident = nc.dram_tensor("ident", [P, P], bf16, kind="ExternalInput").ap()
    out = nc.dram_tensor("out", [TOKC, D], f32, kind="ExternalOutput").ap()
    with tile.TileContext(nc) as tc:
        _kernel_body(tc, out, x_main, x_halo, wdiag, ident, repeat=repeat)
    nc.compile()
    _cache[("nc", repeat)] = nc
    return nc


def _make_in_maps(x, norm_weight, conv_weight):
    bf = ml_dtypes.bfloat16
    w = (conv_weight[:, 0, :] * norm_weight[:, None]).astype(np.float32)  # [D, K]
    wdiag = np.zeros((NCH, KSZ, P, P), np.float32)
    for c in range(NCH):
        for k in range(KSZ):
            np.fill_diagonal(wdiag[c, k], w[c * P:(c + 1) * P, k])
    wdiag = np.ascontiguousarray(wdiag.transpose(2, 0, 1, 3)).astype(bf)
    ident = np.eye(P, dtype=bf)
    zero_halo = np.zeros((PAD, D), np.float32)
    in_maps = []
    for core in range(N_CORES):
        b, h = core // 2, core % 2
        xm = np.ascontiguousarray(x[b, h * TOKC:(h + 1) * TOKC, :])
        xh = (
            np.ascontiguousarray(x[b, TOKC - PAD:TOKC, :]) if h == 1 else zero_halo
        )
        in_maps.append({"x_main": xm, "x_halo": xh, "wdiag": wdiag, "ident": ident})
    return in_maps


def _run(inputs, trace=False, repeat=1):
    from concourse import bass_utils

    nc = _build(repeat)
    in_maps = _make_in_maps(
        np.asarray(inputs["x"]),
        np.asarray(inputs["norm_weight"]),
        np.asarray(inputs["conv_weight"]),
    )
    kw = {}
    if trace:
        kw = dict(trace=True, trace_cores=list(range(N_CORES)))
    res = bass_utils.run_bass_kernel_spmd(
        nc, in_maps, core_ids=list(range(N_CORES)), **kw
    )
    outs = [res.results[i]["out"] for i in range(N_CORES)]
    full = np.stack(
        [np.concatenate([outs[2 * b], outs[2 * b + 1]], axis=0) for b in range(B)]
    )
    return full, res


def kernel(**inputs):
    full, _ = _run(inputs, trace=False)
    return full
